# revision 17
# baseline (speedup 1.0000x reference)
"""Trainium2 Bass kernel for ConstructAdjMatrixWithHomogeneous.

out = I + D^-1/2 @ adj @ D^-1/2,  adj = [[C, A], [A^T, Dd]],
C = filtered_cell_kernel [4000,4000], Dd = filtered_drug_sim [4000,4000],
A = original_cell_drug_adj [4000,4000]; deg = rowsum(adj)+eps, d = deg**-0.5.

Sharding (8 cores): overlapping 512-row slices of each input matrix —
row starts R0 = [0, 512, ..., 3072, 3488]; core 7 overlaps core 6 by 96
rows so every slice is exactly 512 = 4x128 rows. Core 7's A slice has
its 96 overlap rows zeroed host-side so the column-sum partial is not
double counted; other overlap outputs are discarded at assembly.

Output magnitudes are ~2.5e-4 off the diagonal (deg ~ 4000), so reduced
precision is essentially free accuracy-wise: the scaling pass moves
bf16 (~1e-6 end-to-end relative error) and the degree pass reads fp8
e4m3 (degree error ~1.5e-3 relative, which perturbs outputs by only
~4e-7 absolute).  The host packs each core's three bands into ONE
array cad = [C_band | A_band | D_band] of shape [512, 12000], in both
bf16 (launch 2) and fp8 (launch 1) flavors, moved as single large DMAs
(near peak HBM efficiency, minimal instruction count).

Launch 1 (degree partials, fp8): per 128-row band, one 1.5 MB load,
three row-sum reductions (f32 accum) on the C/A/D slices — split
across DVE (reduce) and ACT (activation accum_out), which both run ~1
elem/cycle/lane, so neither engine exceeds the DMA time — and 8 PE
ones-matmuls accumulating partial column sums of A into PSUM.  Host
gathers the 8000-long degree vector ("all-gather"), computes
d = (deg+eps)**-0.5.

Launch 2 (scaling): per band, one 3 MB load, two fused DVE
scalar_tensor_tensor ops computing (x * d_row) * d_col in one pass
(row scale is a per-partition f32 scalar, column scale a broadcast bf16
vector [d_cell | d_drug | d_drug]), one 3 MB store of [scaledC |
scaledA | scaledD].  No tensor-engine transpose: the bottom-left block
A^T is exactly the transpose of the top-right block, so the host
mirrors it during assembly.  The +identity and the 8000 diagonal
entries are fixed up host-side in f32, overwriting the device values,
so bf16's coarse spacing near 1.0 never shows up in the output.

DMA discipline: HWDGE only, loads on the sync ring, dependent stores on
the scalar ring so a load never queues behind a store that waits on
compute.
"""
import sys

sys.path.insert(0, "/opt/trn_rl_repo")

import contextlib
import json
import numpy as np
import ml_dtypes

import concourse.bass as bass
import concourse.mybir as mybir
import concourse.tile as tile
import concourse.bass2jax as bass2jax
from concourse.bass_utils import run_bass_kernel_spmd, compile_bir_kernel

F32 = mybir.dt.float32
BF16 = mybir.dt.bfloat16
FP8 = mybir.dt.float8e4
NPBF16 = ml_dtypes.bfloat16
NPFP8 = ml_dtypes.float8_e4m3
NCORES = 8
PB = 128               # partition band size
NBAND = 4
CR = PB * NBAND        # 512 rows of each matrix per core (overlapping)
NMAT = 4000
N = 8000
W = 3 * NMAT           # combined row width: [C | A | D]
EPS = np.float32(1e-9)
R0 = [512 * k for k in range(7)] + [NMAT - CR]          # slice starts
OWN = [(512 * k, 512 * (k + 1)) for k in range(7)] + [(3584, 4000)]

# ---------------------------------------------------------------------------
# Walrus workaround: this toolchain only supports ONE sync-wait condition per
# instruction ("Too many sync wait commands" in CoreV3GenImpl otherwise).
# Split any instruction carrying >1 waits into preceding NoOps, 1 wait each.
# ---------------------------------------------------------------------------
_MAXW = 1


def _split_waits_bytes(bir_bytes):
    bir = json.loads(bir_bytes)
    n_new = 0
    for fn in bir["functions"]:
        for blk in fn["blocks"]:
            insts = blk.get("instructions", [])
            out = []
            for ins in insts:
                si = ins.get("sync_info") or {}
                waits = si.get("on_wait") or []
                while len(waits) > _MAXW:
                    chunk, waits = waits[:_MAXW], waits[_MAXW:]
                    n_new += 1
                    out.append({
                        "name": ins["name"] + f"_ws{n_new}",
                        "opcode": "NoOp",
                        "engine": ins["engine"],
                        "ins": [], "outs": [],
                        "sync_info": {"on_update": [], "on_wait": chunk},
                    })
                si["on_wait"] = waits
                ins["sync_info"] = si
                out.append(ins)
            blk["instructions"] = out
    return json.dumps(bir).encode()


def _patched_compile_bir_kernel(bir_json, tmpdir, neff_name="file.neff"):
    return compile_bir_kernel(_split_waits_bytes(bir_json), tmpdir,
                              neff_name=neff_name)


bass2jax.compile_bir_kernel = _patched_compile_bir_kernel


def _rep_ctx(tc, reps):
    # reps>1 is a timing-only mode: run the body in a hardware loop.
    return tc.For_i(0, reps, 1) if reps > 1 else contextlib.nullcontext()


# ---------------------------------------------------------------------------
# Launch 1: degree partials (fp8 e4m3 in, f32 out).  fp8 halves the load
# bytes (row/col sums tolerate ~6% elementwise quantization: the resulting
# degree error is ~1.5e-3 relative, which perturbs the output values by
# ~4e-7 absolute — far inside the gate).
#   rs [512,3]: row sums of this core's C/A/Dd rows (cols 0/1/2),
#   cs_a [1,4000]: partial column sums of this core's A rows.
# Row-sum work (12 x [128,4000] @ ~1 elem/cycle/lane) is the bottleneck at
# fp8 load rates, so it is split evenly: DVE and ACT alternate taking 2-of-3
# sums per band (6 ops each per pass); the PSUM->SBUF colsum copies are
# split 4/4 between DVE and ACT (GPSIMD cannot read PSUM).
# ---------------------------------------------------------------------------
def _build_l1(reps=1):
    nc = bass.Bass()
    cad = nc.declare_dram_parameter("cad", [CR, W], FP8, isOutput=False)
    rs = nc.declare_dram_parameter("rs", [CR, 3], F32, isOutput=True)
    cs_a = nc.declare_dram_parameter("cs_a", [1, NMAT], F32, isOutput=True)

    NCHUNK = 8
    CW = NMAT // NCHUNK  # 500
    Copy = mybir.ActivationFunctionType.Copy

    with tile.TileContext(nc) as tc:
        with (
            tc.tile_pool(name="inp", bufs=4) as inp,
            tc.tile_pool(name="red", bufs=4) as red,
            tc.tile_pool(name="scr", bufs=2) as scr,
            tc.tile_pool(name="csout", bufs=1) as csout,
            tc.tile_pool(name="const", bufs=1) as const,
            tc.tile_pool(name="ps", bufs=1, space="PSUM") as ps,
        ):
            ones = const.tile([PB, 1], FP8)
            nc.gpsimd.memset(ones[:], 1.0)

            pscs = [ps.tile([1, CW], F32, tag=f"cs{j}", name=f"cs{j}")
                    for j in range(NCHUNK)]

            with _rep_ctx(tc, reps):
                for b in range(NBAND):
                    t = inp.tile([PB, W], FP8, tag="t", name="t")
                    nc.sync.dma_start(t[:], cad[b * PB:(b + 1) * PB, :])
                    r = red.tile([PB, 3], F32, tag="r", name="r")
                    s = scr.tile([PB, NMAT], FP8, tag="s", name="s")
                    dve_m = (0, 1) if b % 2 == 0 else (0,)
                    for m in range(3):
                        if m in dve_m:
                            nc.vector.reduce_sum(
                                r[:, m:m + 1], t[:, m * NMAT:(m + 1) * NMAT],
                                axis=mybir.AxisListType.X)
                        else:
                            nc.scalar.activation(
                                s[:], t[:, m * NMAT:(m + 1) * NMAT], Copy,
                                accum_out=r[:, m:m + 1])
                    nc.scalar.dma_start(rs[b * PB:(b + 1) * PB, :], r[:])
                    for j in range(NCHUNK):
                        nc.tensor.matmul(
                            pscs[j][:],
                            ones[:],
                            t[:, NMAT + j * CW:NMAT + (j + 1) * CW],
                            start=(b == 0),
                            stop=(b == NBAND - 1),
                        )
                cst = csout.tile([1, NMAT], F32, tag="cs", name="cst")
                for j in range(NCHUNK):
                    if j < 4:
                        nc.vector.tensor_copy(cst[:, j * CW:(j + 1) * CW],
                                              pscs[j][:])
                    else:
                        nc.scalar.copy(cst[:, j * CW:(j + 1) * CW], pscs[j][:])
                nc.scalar.dma_start(cs_a[:], cst[:])
    return nc


# ---------------------------------------------------------------------------
# Launch 2: scaling (all bf16 data).
# Inputs: cad [512,12000] bf16; drow [128,8] f32 (col b = d of cell band
#   b rows, col 4+b = d of drug band b rows); dbc [128,12000] bf16
#   (= [d_cell | d_drug | d_drug] broadcast along partitions).
# Output: oall [512,12000] bf16 = [scaledC | scaledA | scaledD].
# ---------------------------------------------------------------------------
def _build_l2(reps=1):
    nc = bass.Bass()
    cad = nc.declare_dram_parameter("cad", [CR, W], BF16, isOutput=False)
    drow = nc.declare_dram_parameter("drow", [PB, 2 * NBAND], F32, isOutput=False)
    dbc = nc.declare_dram_parameter("dbc", [PB, W], BF16, isOutput=False)
    oall = nc.declare_dram_parameter("oall", [CR, W], BF16, isOutput=True)

    mult = mybir.AluOpType.mult

    with tile.TileContext(nc) as tc:
        with (
            tc.tile_pool(name="const", bufs=1) as const,
            tc.tile_pool(name="inp", bufs=4) as inp,
        ):
            # const loads ride the (initially idle) scalar ring so they never
            # head-block the first band load on the sync ring
            dbct = const.tile([PB, W], BF16)
            nc.scalar.dma_start(dbct[:], dbc[:])
            drt = const.tile([PB, 2 * NBAND], F32)
            nc.scalar.dma_start(drt[:], drow[:])

            with _rep_ctx(tc, reps):
                for b in range(NBAND):
                    t = inp.tile([PB, W], BF16, tag="t", name="t")
                    nc.sync.dma_start(t[:], cad[b * PB:(b + 1) * PB, :])
                    # C and A rows are cell rows: scale by d_cell[row]
                    nc.vector.scalar_tensor_tensor(
                        t[:, 0:2 * NMAT], t[:, 0:2 * NMAT],
                        drt[:, b:b + 1], dbct[:, 0:2 * NMAT],
                        op0=mult, op1=mult)
                    # D rows are drug rows: scale by d_drug[row]
                    nc.vector.scalar_tensor_tensor(
                        t[:, 2 * NMAT:], t[:, 2 * NMAT:],
                        drt[:, NBAND + b:NBAND + b + 1], dbct[:, 2 * NMAT:],
                        op0=mult, op1=mult)
                    nc.scalar.dma_start(oall[b * PB:(b + 1) * PB, :], t[:])
    return nc


_programs_cache = {}


def _programs():
    if "l1" not in _programs_cache:
        _programs_cache["l1"] = _build_l1()
        _programs_cache["l2"] = _build_l2()
    return _programs_cache["l1"], _programs_cache["l2"]


def kernel(filtered_cell_kernel, filtered_drug_sim, original_cell_drug_adj,
           enable_homogeneous_graph):
    C = np.asarray(filtered_cell_kernel, dtype=np.float32)
    D = np.asarray(filtered_drug_sim, dtype=np.float32)
    A = np.asarray(original_cell_drug_adj, dtype=np.float32)
    enable = int(np.asarray(enable_homogeneous_graph))
    if not enable:
        C = np.zeros_like(C)
        D = np.zeros_like(D)

    l1, l2 = _programs()
    cores = list(range(NCORES))

    # Pack [C_band | A_band | D_band] per core: bf16 for L2, fp8 for L1.
    cad = []
    cad8 = []
    for k in range(NCORES):
        r0 = R0[k]
        blk = np.empty((CR, W), dtype=NPBF16)
        blk[:, 0:NMAT] = C[r0:r0 + CR]
        blk[:, NMAT:2 * NMAT] = A[r0:r0 + CR]
        blk[:, 2 * NMAT:] = D[r0:r0 + CR]
        if k == 7:
            blk[: OWN[7][0] - R0[7], NMAT:2 * NMAT] = 0.0  # zero overlap rows
        cad.append(blk)
        cad8.append(blk.astype(NPFP8))

    in1 = [{"cad": cad8[k]} for k in range(NCORES)]
    r1 = run_bass_kernel_spmd(l1, in1, core_ids=cores).results

    deg = np.empty(N, dtype=np.float32)
    cs_a = np.zeros(NMAT, dtype=np.float32)
    for k in range(NCORES):
        s, e = OWN[k]
        lo = s - R0[k]
        deg[s:e] = (r1[k]["rs"][lo:lo + (e - s), 0]
                    + r1[k]["rs"][lo:lo + (e - s), 1])
        deg[NMAT + s:NMAT + e] = r1[k]["rs"][lo:lo + (e - s), 2]
        cs_a += r1[k]["cs_a"][0]
    deg[NMAT:] += cs_a

    total = float(deg.astype(np.float64).sum())
    if total == 0.0:
        return np.eye(N, dtype=np.float32)

    degp = (deg + EPS).astype(np.float32)
    d = degp ** np.float32(-0.5)
    d = np.where(np.isinf(d), np.float32(0.0), d).astype(np.float32)

    d16 = d.astype(NPBF16)
    dbc_row = np.concatenate([d16, d16[NMAT:]])          # [d_cell|d_drug|d_drug]
    dbc = np.ascontiguousarray(np.broadcast_to(dbc_row, (PB, W)))
    in2 = []
    for k in range(NCORES):
        r0 = R0[k]
        drow_k = np.concatenate([d[r0:r0 + CR], d[NMAT + r0:NMAT + r0 + CR]])
        drow = np.ascontiguousarray(drow_k.reshape(2 * NBAND, PB).T)
        in2.append({"cad": cad[k], "drow": drow, "dbc": dbc})

    r2 = run_bass_kernel_spmd(l2, in2, core_ids=cores).results

    out = np.empty((N, N), dtype=np.float32)
    for k in range(NCORES):
        s, e = OWN[k]
        lo = s - R0[k]
        out[s:e, :] = r2[k]["oall"][lo:lo + (e - s), 0:2 * NMAT]
        out[NMAT + s:NMAT + e, NMAT:] = r2[k]["oall"][lo:lo + (e - s), 2 * NMAT:]
    # bottom-left block is exactly the transpose of the top-right block
    out[NMAT:, :NMAT] = out[:NMAT, NMAT:].T
    # identity + exact f32 diagonal (device diagonal values are overwritten)
    idx = np.arange(NMAT)
    out[idx, idx] = np.float32(1.0) + d[:NMAT] * d[:NMAT] * np.diagonal(C)
    out[NMAT + idx, NMAT + idx] = (np.float32(1.0)
                                   + d[NMAT:] * d[NMAT:] * np.diagonal(D))
    return out
